# revision 5
# baseline (speedup 1.0000x reference)
"""Trainium2 Bass kernel for 2-layer GAT (nn_GAT_59133109732231).

Self-contained: kernel(**inputs) -> np.ndarray [100000, 2] float32.

Distribution (8 NeuronCores, SPMD):
  - nodes permuted so core c owns S_SC=120 superchunks x 128 output rows
    (row 127 of each superchunk = trash row for pad edges).
  - a superchunk owns <=127 dst nodes and all their in-edges (self-loops
    handled separately, locally), packed into 4 segments of SEG=256 edge
    slots keyed by src quadrant (= src owner core pair).
  - per layer: transform local nodes with augmented weights
    [W | W@a_src | W@a_dst] (one matmul yields h + both attention dots),
    write bf16 records [h0|1|h1|1|asrc|pad] (the 1.0 columns make the
    softmax denominator fall out of the aggregation matmul), ONE 8-rank
    AllGather -> full record table; per superchunk dma_gather 768B records
    by int16 row index and aggregate with one-hot matmuls in PSUM.
"""
import os
import sys

import numpy as np
import ml_dtypes

for _p in ("/opt/trn_rl_repo", "/root/.axon_site/_ro/trn_rl_repo"):
    if os.path.isdir(_p) and _p not in sys.path:
        sys.path.append(_p)

N = 100000
NCORES = 8
S_SC = 120
SEG = 256
SLOTS_SC = 4 * SEG
ROWS_CORE = S_SC * 128          # 15360
QROWS = 2 * ROWS_CORE           # 30720 rows per quadrant table
GRP = 8
NGRP = S_SC // GRP              # 15
NEG_SLOPE = 0.2
REC = 384                       # record cols (bf16) = 768 B
EIDX_COLS = NGRP * 4 * 128      # 7680

bf16 = ml_dtypes.bfloat16


# ----------------------------------------------------------------- host prep
def build_plan(edge_index):
    edge_index = np.asarray(edge_index)
    src = edge_index[0].astype(np.int64)
    dst = edge_index[1].astype(np.int64)

    deg = np.bincount(dst, minlength=N)
    order = np.argsort(-deg, kind="stable")
    owner = np.empty(N, dtype=np.int32)
    snake = np.tile(np.concatenate([np.arange(8), np.arange(7, -1, -1)]), N // 16 + 1)[:N]
    owner[order] = snake.astype(np.int32)

    e_q = (owner[src] // 2).astype(np.int32)
    qd = np.zeros((N, 4), dtype=np.int32)
    np.add.at(qd, (dst, e_q), 1)

    sc_of = np.empty(N, dtype=np.int32)
    row_of = np.empty(N, dtype=np.int32)
    for c in range(8):
        nodes = np.where(owner == c)[0]
        nodes = nodes[np.argsort(-deg[nodes], kind="stable")]
        loads = np.zeros((S_SC, 4), dtype=np.int32)
        counts = np.zeros(S_SC, dtype=np.int32)
        tot = np.zeros(S_SC, dtype=np.int32)
        big = 1.0e9
        for n in nodes:
            after = loads + qd[n][None, :]
            ok = (after <= SEG).all(axis=1) & (counts < 127)
            key = after.max(axis=1).astype(np.float64) + tot * 1e-6 + (~ok) * big
            k = int(np.argmin(key))
            assert ok[k], "packing failed"
            sc_of[n] = k
            row_of[n] = counts[k]
            counts[k] += 1
            loads[k] += qd[n]
            tot[k] += deg[n]
    rowq_of = ((owner % 2) * ROWS_CORE + sc_of * 128 + row_of).astype(np.int32)

    e_core = owner[dst]
    e_sc = sc_of[dst]
    e_rowq = rowq_of[src]
    e_dloc = row_of[dst]

    plans = []
    for c in range(8):
        eidx = np.zeros((S_SC, 4, SEG), dtype=np.int16)
        dloc = np.full((S_SC, 4, SEG), 127, dtype=np.int32)
        m = e_core == c
        sc_c, q_c, rq_c, dl_c = e_sc[m], e_q[m], e_rowq[m], e_dloc[m]
        o = np.lexsort((q_c, sc_c))
        sc_c, q_c, rq_c, dl_c = sc_c[o], q_c[o], rq_c[o], dl_c[o]
        key = sc_c * 4 + q_c
        pos = np.arange(len(key)) - np.searchsorted(key, key, side="left")
        assert pos.max() < SEG
        eidx[sc_c, q_c, pos] = rq_c.astype(np.int16)
        dloc[sc_c, q_c, pos] = dl_c
        plans.append(dict(eidx=eidx, dloc=dloc))
    return dict(owner=owner, sc_of=sc_of, row_of=row_of, plans=plans)


def make_core_inputs(plan, inputs):
    x = np.asarray(inputs["x"], dtype=np.float32)

    def amat(a):
        a = np.asarray(a, dtype=np.float32)
        m = np.zeros((256, 2), dtype=np.float32)
        m[0:128, 0] = a[0]
        m[128:256, 1] = a[1]
        return m

    W1 = np.asarray(inputs["W1"], dtype=np.float32)
    W2 = np.asarray(inputs["W2"], dtype=np.float32)
    W1aug = np.concatenate(
        [W1, W1 @ amat(inputs["a_src1"]), W1 @ amat(inputs["a_dst1"])], axis=1)
    W2aug = np.concatenate(
        [W2, W2 @ amat(inputs["a_src2"]), W2 @ amat(inputs["a_dst2"])], axis=1)

    rep = lambda v, d: np.broadcast_to(
        np.asarray(v, dtype=np.float32)[None, :], (128, d)).copy()
    shared = dict(
        w1aug=W1aug.astype(bf16),
        w2aug=W2aug.astype(bf16),
        wp1=np.asarray(inputs["Wp1"], dtype=np.float32).astype(bf16),
        wp2=np.asarray(inputs["Wp2"], dtype=np.float32).astype(bf16),
        b1=rep(inputs["b1"], 256), b2=rep(inputs["b2"], 256),
        bp1=rep(inputs["bp1"], 128), bp2=rep(inputs["bp2"], 2),
        ident=np.eye(128, dtype=np.float32).astype(bf16),
        iotam=np.broadcast_to(np.arange(128, dtype=np.float32).astype(bf16)[None, None, :],
                              (128, 8, 128)).copy(),
        iotac=np.arange(128, dtype=np.float32).astype(bf16)[:, None].copy(),
    )

    owner, sc_of, row_of = plan["owner"], plan["sc_of"], plan["row_of"]
    cores = []
    for c in range(8):
        xp = np.zeros((ROWS_CORE, 128), dtype=np.float32)
        nodes = np.where(owner == c)[0]
        xp[sc_of[nodes] * 128 + row_of[nodes]] = x[nodes]
        p = plan["plans"][c]
        eidx_w = np.zeros((128, EIDX_COLS), dtype=np.int16)
        for g in range(NGRP):
            for q in range(4):
                idxs = p["eidx"][g * GRP:(g + 1) * GRP, q, :].reshape(-1)
                w = idxs.reshape(128, 16).T
                col0 = (g * 4 + q) * 128
                eidx_w[:, col0:col0 + 128] = np.tile(w, (8, 1))
        dl = p["dloc"].reshape(S_SC, 8, 128)  # [sc, chunk j, slot]
        dloc_b = np.ascontiguousarray(
            dl.transpose(2, 0, 1).reshape(128, S_SC * 8)).astype(np.float32).astype(bf16)
        dlocT = np.broadcast_to(
            dl.reshape(1, S_SC * 8 * 128).astype(np.float32).astype(bf16),
            (128, S_SC * 1024)).copy()
        cores.append(dict(xp=xp, eidx=eidx_w, dstloc=dloc_b, dstlocT=dlocT))
    return cores, shared


# -------------------------------------------------------------- bass program
def build_nc():
    import concourse.bass as bass
    import concourse.bacc as bacc
    import concourse.mybir as mybir
    import concourse.tile as tile

    F32, BF, I16 = mybir.dt.float32, mybir.dt.bfloat16, mybir.dt.int16
    AF = mybir.ActivationFunctionType
    ALU = mybir.AluOpType

    nc = bacc.Bacc("TRN2", target_bir_lowering=False, debug=False, num_devices=8)

    din = {}
    for name, shape, dt in [
        ("xp", [ROWS_CORE, 128], F32),
        ("eidx", [128, EIDX_COLS], I16),
        ("dstloc", [128, S_SC * 8], BF),
        ("dstlocT", [128, S_SC * 1024], BF),
        ("w1aug", [128, 260], BF),
        ("w2aug", [256, 260], BF),
        ("wp1", [256, 128], BF),
        ("wp2", [128, 2], BF),
        ("b1", [128, 256], F32), ("b2", [128, 256], F32),
        ("bp1", [128, 128], F32), ("bp2", [128, 2], F32),
        ("ident", [128, 128], BF),
        ("iotam", [128, 8, 128], BF),
        ("iotac", [128, 1], BF),
    ]:
        din[name] = nc.dram_tensor(name, shape, dt, kind="ExternalInput")
    y_d = nc.dram_tensor("y", [ROWS_CORE, 2], F32, kind="ExternalOutput")
    shard = nc.dram_tensor("shard", [ROWS_CORE, REC], BF, kind="Internal")
    table = nc.dram_tensor("table", [8 * ROWS_CORE, REC], BF, kind="Internal",
                           addr_space="Shared")
    out1 = nc.dram_tensor("out1", [ROWS_CORE, 256], BF, kind="Internal")

    with tile.TileContext(nc) as tc:
        import contextlib
        ctx = contextlib.ExitStack()
        with ctx:
            pp = ctx.enter_context(tc.tile_pool(name="pp", bufs=1))
            sb = ctx.enter_context(tc.tile_pool(name="sb", bufs=3))
            gp = ctx.enter_context(tc.tile_pool(name="gp", bufs=2))
            ps = ctx.enter_context(tc.tile_pool(name="ps", bufs=2, space="PSUM"))
            ps1 = ctx.enter_context(tc.tile_pool(name="ps1", bufs=1, space="PSUM"))

            # persistent SBUF
            P = {}
            for name in ("eidx", "dstloc", "w1aug", "wp2", "b1", "b2",
                         "bp1", "bp2", "ident", "iotam", "iotac"):
                t = pp.tile(list(din[name].shape), din[name].dtype, tag=f"p_{name}")
                nc.sync.dma_start(t[:], din[name].ap())
                P[name] = t
            w2s = pp.tile([128, 2, 260], BF, tag="p_w2")
            nc.sync.dma_start(w2s[:, 0, :], din["w2aug"].ap()[0:128, :])
            nc.sync.dma_start(w2s[:, 1, :], din["w2aug"].ap()[128:256, :])
            wp1s = pp.tile([128, 2, 128], BF, tag="p_wp1")
            nc.sync.dma_start(wp1s[:, 0, :], din["wp1"].ap()[0:128, :])
            nc.sync.dma_start(wp1s[:, 1, :], din["wp1"].ap()[128:256, :])
            aa = pp.tile([128, S_SC * 4], F32, tag="p_aa")   # asrc|adst per sc

            for layer in range(2):
                nin = 1 if layer == 0 else 2
                bias = P["b1"] if layer == 0 else P["b2"]

                # ---- phase T: transform + record build
                for k in range(S_SC):
                    xt = sb.tile([128, nin * 128], BF, tag="xt")
                    if layer == 0:
                        nc.gpsimd.dma_start(xt[:, 0:128],
                                            din["xp"].ap()[k * 128:(k + 1) * 128, :])
                    else:
                        nc.sync.dma_start(xt[:], out1.ap()[k * 128:(k + 1) * 128, :])
                    ph = ps.tile([128, 260], F32, tag="ph")
                    for chx in range(nin):
                        pt = ps1.tile([128, 128], BF, tag="ptx")
                        nc.tensor.transpose(pt[:], xt[:, chx * 128:(chx + 1) * 128],
                                            P["ident"][:])
                        xT = sb.tile([128, 128], BF, tag="xT")
                        nc.scalar.activation(xT[:], pt[:], AF.Copy)
                        w_rhs = P["w1aug"][:] if layer == 0 else w2s[:, chx, :]
                        nc.tensor.matmul(ph[:], lhsT=xT[:], rhs=w_rhs,
                                         start=(chx == 0), stop=(chx == nin - 1))
                    rec = sb.tile([128, REC], BF, tag="rec")
                    nc.scalar.activation(rec[:, 0:128], ph[:, 0:128], AF.Copy)
                    nc.scalar.activation(rec[:, 129:257], ph[:, 128:256], AF.Copy)
                    nc.vector.tensor_copy(rec[:, 258:260], ph[:, 256:258])
                    nc.vector.memset(rec[:, 128:129], 1.0)
                    nc.vector.memset(rec[:, 257:258], 1.0)
                    nc.vector.memset(rec[:, 260:REC], 0.0)
                    nc.vector.tensor_copy(aa[:, 4 * k:4 * k + 4], ph[:, 256:260])
                    nc.sync.dma_start(shard.ap()[k * 128:(k + 1) * 128, :], rec[:])

                # ---- allgather record table
                nc.gpsimd.collective_compute(
                    "AllGather", ALU.bypass,
                    replica_groups=[list(range(8))],
                    ins=[shard.ap()], outs=[table.ap()])

                # ---- phase E: gather + aggregate
                for g in range(NGRP):
                    dT = gp.tile([128, GRP * 1024], BF, tag="dT")
                    nc.sync.dma_start(
                        dT[:], din["dstlocT"].ap()[:, g * GRP * 1024:(g + 1) * GRP * 1024])
                    gts = []
                    for q in range(4):
                        gt = gp.tile([128, 16, REC], BF, tag=f"gt{q}")
                        nc.gpsimd.dma_gather(
                            gt[:],
                            table.ap()[QROWS * q:QROWS * (q + 1), :],
                            P["eidx"][:, (g * 4 + q) * 128:(g * 4 + q + 1) * 128],
                            2048, 2048, REC, single_packet=False)
                        gts.append(gt)
                    for jl in range(GRP):
                        k = g * GRP + jl
                        srec = sb.tile([128, REC], BF, tag="srec")
                        nc.sync.dma_start(srec[:], shard.ap()[k * 128:(k + 1) * 128, :])
                        ls = sb.tile([128, 2], F32, tag="ls")
                        nc.vector.tensor_add(ls[:], aa[:, 4 * k:4 * k + 2],
                                             aa[:, 4 * k + 2:4 * k + 4])
                        nc.scalar.activation(ls[:], ls[:], AF.Lrelu, alpha=NEG_SLOPE)
                        ws = sb.tile([128, 2], F32, tag="ws")
                        nc.scalar.activation(ws[:], ls[:], AF.Exp)
                        rhs_s = sb.tile([128, 258], BF, tag="rhss")
                        nc.scalar.activation(rhs_s[:, 0:129], srec[:, 0:129],
                                             AF.Copy, scale=ws[:, 0:1])
                        nc.vector.tensor_scalar_mul(rhs_s[:, 129:258],
                                                    srec[:, 129:258], ws[:, 1:2])
                        po = ps.tile([128, 258], F32, tag="po")
                        nc.tensor.matmul(po[:], lhsT=P["ident"][:], rhs=rhs_s[:],
                                         start=True, stop=False)
                        # one-hot (dst-partition orientation) + adst expand
                        oT = sb.tile([128, 1024], BF, tag="oT")
                        nc.vector.tensor_tensor(
                            oT[:], dT[:, jl * 1024:(jl + 1) * 1024],
                            P["iotac"][:].to_broadcast([128, 1024]), ALU.is_equal)
                        adb = sb.tile([128, 2], BF, tag="adb")
                        nc.vector.tensor_copy(adb[:], aa[:, 4 * k + 2:4 * k + 4])
                        pae = ps1.tile([128, 8, 2], F32, tag="pae")
                        for j in range(8):
                            nc.tensor.matmul(pae[:, j, :],
                                             lhsT=oT[:, j * 128:(j + 1) * 128],
                                             rhs=adb[:], start=True, stop=True)
                        # logits -> edge weights
                        asr = sb.tile([128, 8, 2], F32, tag="asr")
                        for q in range(4):
                            nc.vector.tensor_copy(
                                asr[:, 2 * q:2 * q + 2, :],
                                gts[q][:, 2 * jl:2 * jl + 2, 258:260])
                        lg = sb.tile([128, 8, 2], F32, tag="lg")
                        nc.vector.tensor_add(lg[:], asr[:], pae[:])
                        nc.scalar.activation(lg[:], lg[:], AF.Lrelu, alpha=NEG_SLOPE)
                        we = sb.tile([128, 8, 2], F32, tag="we")
                        nc.scalar.activation(we[:], lg[:], AF.Exp)
                        # one-hot (edge-partition orientation)
                        oh = sb.tile([128, 8, 128], BF, tag="oh")
                        nc.vector.tensor_tensor(
                            oh[:],
                            P["dstloc"][:, 8 * k:8 * k + 8].to_broadcast([128, 8, 128]),
                            P["iotam"][:], ALU.is_equal)
                        rhs_e = sb.tile([128, 8, 258], BF, tag="rhse")
                        for j in range(8):
                            q, jj = j // 2, j % 2
                            col = 2 * jl + jj
                            nc.scalar.activation(rhs_e[:, j, 0:129],
                                                 gts[q][:, col, 0:129],
                                                 AF.Copy, scale=we[:, j, 0:1])
                            nc.vector.tensor_scalar_mul(rhs_e[:, j, 129:258],
                                                        gts[q][:, col, 129:258],
                                                        we[:, j, 1:2])
                        for j in range(8):
                            nc.tensor.matmul(po[:], lhsT=oh[:, j, :],
                                             rhs=rhs_e[:, j, :],
                                             start=False, stop=(j == 7))
                        # finalize
                        den = sb.tile([128, 2], F32, tag="den")
                        nc.vector.tensor_copy(den[:, 0:1], po[:, 128:129])
                        nc.vector.tensor_copy(den[:, 1:2], po[:, 257:258])
                        nc.vector.tensor_scalar_add(den[:], den[:], 1e-16)
                        nc.vector.reciprocal(den[:], den[:])
                        of = sb.tile([128, 256], F32, tag="of")
                        nc.vector.tensor_scalar_mul(of[:, 0:128], po[:, 0:128],
                                                    den[:, 0:1])
                        nc.vector.tensor_scalar_mul(of[:, 128:256], po[:, 129:257],
                                                    den[:, 1:2])
                        nc.vector.tensor_add(of[:], of[:], bias[:])
                        o2 = sb.tile([128, 256], BF, tag="o2")
                        nc.scalar.activation(o2[:], of[:], AF.Relu)
                        if layer == 0:
                            nc.sync.dma_start(out1.ap()[k * 128:(k + 1) * 128, :],
                                              o2[:])
                        else:
                            hT = sb.tile([128, 2, 128], BF, tag="hT")
                            for r in range(2):
                                ptr = ps1.tile([128, 128], BF, tag="ptx")
                                nc.tensor.transpose(ptr[:],
                                                    o2[:, r * 128:(r + 1) * 128],
                                                    P["ident"][:])
                                nc.scalar.activation(hT[:, r, :], ptr[:], AF.Copy)
                            pm = ps1.tile([128, 128], F32, tag="pm2")
                            for r in range(2):
                                nc.tensor.matmul(pm[:], lhsT=hT[:, r, :],
                                                 rhs=wp1s[:, r, :],
                                                 start=(r == 0), stop=(r == 1))
                            t2 = sb.tile([128, 128], BF, tag="t2")
                            nc.vector.tensor_add(t2[:], pm[:], P["bp1"][:])
                            pt2 = ps1.tile([128, 128], BF, tag="ptx")
                            nc.tensor.transpose(pt2[:], t2[:], P["ident"][:])
                            t2T = sb.tile([128, 128], BF, tag="t2T")
                            nc.scalar.activation(t2T[:], pt2[:], AF.Copy)
                            pyy = ps1.tile([128, 2], F32, tag="pm2")
                            nc.tensor.matmul(pyy[:], lhsT=t2T[:], rhs=P["wp2"][:],
                                             start=True, stop=True)
                            yt = sb.tile([128, 2], F32, tag="yt")
                            nc.vector.tensor_add(yt[:], pyy[:], P["bp2"][:])
                            nc.scalar.activation(yt[:], yt[:], AF.Sigmoid)
                            nc.sync.dma_start(y_d.ap()[k * 128:(k + 1) * 128, :],
                                              yt[:])
    nc.compile()
    return nc


_NC_CACHE = None


def kernel(**inputs):
    global _NC_CACHE
    from concourse.bass_utils import run_bass_kernel_spmd

    plan = build_plan(inputs["edge_index"])
    cores, shared = make_core_inputs(plan, inputs)

    if _NC_CACHE is None:
        _NC_CACHE = build_nc()
    nc = _NC_CACHE

    in_maps = []
    for c in range(8):
        m = dict(shared)
        m.update(cores[c])
        in_maps.append({k: np.ascontiguousarray(v) for k, v in m.items()})

    res = run_bass_kernel_spmd(nc, in_maps, core_ids=list(range(8)))

    owner, sc_of, row_of = plan["owner"], plan["sc_of"], plan["row_of"]
    y = np.zeros((N, 2), dtype=np.float32)
    for c in range(8):
        yc = res.results[c]["y"]
        nodes = np.where(owner == c)[0]
        y[nodes] = yc[sc_of[nodes] * 128 + row_of[nodes]]
    return y
